# revision 4
# baseline (speedup 1.0000x reference)
"""Differential attention (GQA + RoPE) Bass/Tile kernel for 8 TRN2 NeuronCores.

Sharding: tensor-parallel over the 16 query heads (2 per core, kv head c//2),
Wq/Wk/Wv column-sharded per core; attention output exchanged with an on-device
AllToAll into sequence shards; o_proj row-parallel per sequence shard with the
full Wo on every core; host concatenates the 8 row shards.

Layout notes:
 - x is passed transposed (xT [D, S]) so the contraction dim of every
   projection matmul lands on SBUF partitions.
 - Wq/Wk columns are permuted per head so that score-half A occupies SBUF
   rows 0..63 and half B rows 64..127:
     rows  0..31  x0 of rope pairs  0..31   (half A even components)
     rows 32..63  x1 of rope pairs  0..31
     rows 64..95  x0 of rope pairs 32..63   (half B)
     rows 96..127 x1 of rope pairs 32..63
   With halves contiguous, the two K=64 score matmuls run CONCURRENTLY in
   disjoint PE row-groups (tile_position (0,0) / (64,0)) -> ~2x score rate.
 - Softmax is max-free (scores within +-8, far inside fp16/exp range); the
   row sum rides the AV matmul as a 129th rhs column of 2.0 against the
   loaded exp(score)^T stationary tile; the doubled sum's reciprocal carries
   the 0.5 output scale.
 - exp() is issued once per PAIR of score tiles out of a [128, 1024] 2-bank
   PSUM tile, halving the per-instruction ACT overhead.
 - a1 - lam*a2 is folded linearly: out = (u1*inv_r1 - lam*u2*inv_r2) * 0.5.
"""

import numpy as np
from contextlib import ExitStack

import concourse.bacc as bacc
import concourse.tile as tile
from concourse import mybir
from concourse.bass_utils import run_bass_kernel_spmd

S = 2048
D = 2048
H = 16
KV = 4
HD = 128
HALF = 64
NCORES = 8
HPC = H // NCORES      # 2 query heads per core
P = 128
NT = S // P            # 16 tiles of 128 along s/t
NSC = 4                # s-chunks of 512
SCW = 512
DT = D // P            # 16 tiles along contraction dim
SROWS = S // NCORES    # 256 output rows per core
SCALE = 1.0 / 8.0      # 1/sqrt(HALF)
OUT_SCALE = 0.5        # 1 - lambda_init
UG2 = 160              # AV chain stride inside a shared PSUM bank (fp32 elems)

f32 = mybir.dt.float32
f16 = mybir.dt.float16

_CACHE = {}


def _build():
    nc = bacc.Bacc("TRN2", target_bir_lowering=False, debug=False,
                   num_devices=NCORES)
    xT = nc.declare_dram_parameter("xT", [D, S], f16, isOutput=False)
    wall = nc.declare_dram_parameter("wall", [D, 4 * P], f16, isOutput=False)
    wo = nc.declare_dram_parameter("wo", [D, D], f16, isOutput=False)
    cosT = nc.declare_dram_parameter("cosT", [P, S], f16, isOutput=False)
    sinT = nc.declare_dram_parameter("sinT", [P, S], f16, isOutput=False)
    masks = nc.declare_dram_parameter("masks", [P, 4 * SCW], f16, isOutput=False)
    ident = nc.declare_dram_parameter("ident", [P, P], f32, isOutput=False)
    lam = nc.declare_dram_parameter("lam", [1, HPC], f32, isOutput=False)
    o_out = nc.declare_dram_parameter("o_out", [SROWS, D], f32, isOutput=True)

    rg = [list(range(NCORES))]

    with tile.TileContext(nc) as tc, ExitStack() as ctx:
        const = ctx.enter_context(tc.tile_pool(name="const", bufs=1))
        dram = ctx.enter_context(tc.tile_pool(name="dram", bufs=1, space="DRAM"))

        # cos/sin rows follow the repacked channel layout:
        # [c0..31 | c0..31 | c32..63 | c32..63]
        cos_sb = const.tile([P, S], f16)
        nc.gpsimd.dma_start(out=cos_sb[:, :], in_=cosT[:, :])
        sin_sb = const.tile([P, S], f16)
        nc.gpsimd.dma_start(out=sin_sb[:, :], in_=sinT[:, :])
        mask_sb = const.tile([P, 4 * SCW], f16)
        nc.gpsimd.dma_start(out=mask_sb[:, :], in_=masks[:, :])
        id_sb = const.tile([P, P], f32)
        nc.gpsimd.dma_start(out=id_sb[:, :], in_=ident[:, :])
        lam_sb = const.tile([1, HPC], f32)
        nc.gpsimd.dma_start(out=lam_sb[:, :], in_=lam[:, :])
        lam_sig = const.tile([1, HPC], f32)
        nc.scalar.activation(lam_sig[:, :], lam_sb[:, :],
                             mybir.ActivationFunctionType.Sigmoid)
        lam_b = const.tile([P, HPC], f32)
        nc.gpsimd.partition_broadcast(lam_b[:, :], lam_sig[:, :])

        # Persistent per-core tensors: q per head (rope applied, half-A rows
        # 0..63 / half-B rows 64..127), k likewise in kAB, vT (fp32
        # channel-major) and v16 (fp16 t-major for the AV rhs).
        qkvp = ctx.enter_context(tc.tile_pool(name="qkvp", bufs=1))
        qkv = [qkvp.tile([P, S], f16, name=f"qkv{j}") for j in range(2)]
        kAB = qkvp.tile([P, S], f16, name="kAB")
        vT32 = qkvp.tile([P, S], f32)
        # v in t-major fp16, one 136-wide group per t-tile:
        # cols [136jt, 136jt+128) = v, col 136jt+128 = 2.0 -- the rowsum
        # rider; pre-doubled sums make their reciprocal carry the 0.5 scale
        VG = 136
        v16 = qkvp.tile([P, NT * VG], f16)
        nc.vector.memset(v16[:, :].rearrange("p (jt g) -> p jt g", g=VG)
                         [:, :, 128:129], 2.0)

        # ---- Stage 1: fused qkv projection (+RoPE on eviction) ----
        with tc.tile_pool(name="wall_p", bufs=1) as wall_pool, \
             tc.tile_pool(name="xt_p", bufs=2) as xt_pool, \
             tc.tile_pool(name="rtmp", bufs=4) as rtmp, \
             tc.tile_pool(name="qscr", bufs=6) as qscr, \
             tc.tile_pool(name="ps1", bufs=2, space="PSUM") as ps1:
            # weight block split into 4 dt-groups spread over 4 DMA queues so
            # the first group lands quickly and matmuls can start early
            w_sb = wall_pool.tile([P, DT * 4 * P], f16, name="w_sb")
            # interleave w/x dt-groups over the two hwdge queues so group 0
            # of both lands fast and matmuls start after ~1MB of DMA
            wq_engines = (nc.scalar, nc.sync, nc.scalar, nc.sync)
            for g in range(4):
                gdt = slice(g * 4, (g + 1) * 4)
                wq_engines[g].dma_start(
                    out=w_sb[:, g * 4 * 4 * P:(g + 1) * 4 * 4 * P].rearrange(
                        "p (dt j) -> p dt j", dt=4),
                    in_=wall.ap().rearrange("(dt p) j -> p dt j", p=P)
                    [:, gdt, :])

            xq_engines = (nc.sync, nc.scalar, nc.sync, nc.scalar)
            for sc in range(NSC):
                # x chunk also in 4 dt-groups over both queues
                xts = xt_pool.tile([P, DT * SCW], f16, name="xt", tag="xt")
                for g in range(4):
                    gdt = slice(g * 4, (g + 1) * 4)
                    xq_engines[g].dma_start(
                        out=xts[:, g * 4 * SCW:(g + 1) * 4 * SCW].rearrange(
                            "p (dt f) -> p dt f", dt=4),
                        in_=xT[:, sc * SCW:(sc + 1) * SCW].rearrange(
                            "(dt p) f -> p dt f", p=P)[:, gdt, :])
                # 4 j-chains live simultaneously (4 banks x bufs=2); dt-groups
                # interleave so compute starts after the first 1MB arrives
                pst = [ps1.tile([P, SCW], f32, name=f"p{j}", tag=f"p{j}")
                       for j in range(4)]
                for g in range(4):
                    for j in (2, 3, 0, 1):
                        for dt_ in range(4 * g, 4 * g + 4):
                            nc.tensor.matmul(
                                pst[j][:, :],
                                w_sb[:, dt_ * 4 * P + j * P:
                                     dt_ * 4 * P + (j + 1) * P],
                                xts[:, dt_ * SCW:(dt_ + 1) * SCW],
                                start=(dt_ == 0), stop=(dt_ == DT - 1))
                ssl = slice(sc * SCW, (sc + 1) * SCW)
                for j in (2, 0, 1, 3):
                    psum_p = pst[j]
                    tags = ("t0", "t1") if j < 2 else ("kt0", "kt1")
                    t0 = rtmp.tile([P, SCW], f16, name="t0", tag=tags[0])
                    t1 = rtmp.tile([P, SCW], f16, name="t1", tag=tags[1])
                    if j < 3:
                        # evict on the scalar engine so the PSUM bank frees
                        # fast; rope then runs SBUF->SBUF on DVE
                        xsc = qscr.tile([P, SCW], f16, name="xsc", tag="xsc")
                        nc.scalar.copy(xsc[:, :], psum_p[:, :])
                        psum_p = xsc
                        dst = qkv[j] if j < 2 else kAB
                        # rope per half: rows [b..b+32) = x0*cos - x1*sin,
                        # rows [b+32..b+64) = x0*sin + x1*cos
                        for b in (0, 64):
                            r0 = slice(b, b + 32)
                            r1 = slice(b + 32, b + 64)
                            nc.vector.tensor_mul(t0[r0, :], psum_p[r1, :],
                                                 sin_sb[r1, ssl])
                            nc.vector.tensor_mul(dst[r0, ssl], psum_p[r0, :],
                                                 cos_sb[r0, ssl])
                            nc.vector.tensor_sub(dst[r0, ssl], dst[r0, ssl],
                                                 t0[r0, :])
                            nc.vector.tensor_mul(t1[r1, :], psum_p[r0, :],
                                                 sin_sb[r0, ssl])
                            nc.vector.tensor_mul(dst[r1, ssl], psum_p[r1, :],
                                                 cos_sb[r1, ssl])
                            nc.vector.tensor_add(dst[r1, ssl], dst[r1, ssl],
                                                 t1[r1, :])
                    else:
                        nc.scalar.copy(vT32[:, ssl], psum_p[:, :])

        # ---- Stage 1.5: transpose v to t-major fp16 ----
        with tc.tile_pool(name="pst", bufs=1, space="PSUM") as pstp:
            for jt in range(NT):
                ps_t = pstp.tile([P, P], f32, name="ps_vt", tag="vt")
                nc.tensor.transpose(ps_t[:, :],
                                    vT32[:, jt * P:(jt + 1) * P], id_sb[:, :])
                nc.scalar.copy(v16[:, jt * VG:jt * VG + P], ps_t[:, :])

        # prefetch full Wo (fp16) early on the gpsimd DMA queue so the
        # o_proj partials can start the moment the first AllToAll lands
        wo_pool = ctx.enter_context(tc.tile_pool(name="wo_p", bufs=1))
        wos_l = []
        for dc in range(4):
            wos = wo_pool.tile([P, H * SCW], f16, name="wos", tag=f"wos{dc}")
            nc.gpsimd.dma_start(
                out=wos[:, :].rearrange("p (ht f) -> p ht f", ht=H),
                in_=wo[:, dc * SCW:(dc + 1) * SCW].rearrange(
                    "(ht p) f -> p ht f", p=P))
            wos_l.append(wos)

        # ---- Stage 2: differential attention per (head, s-chunk) ----
        attnp = ctx.enter_context(tc.tile_pool(name="attnp", bufs=1))
        attnT = [attnp.tile([P, S], f16, name=f"attnT{h}") for h in range(HPC)]

        # per-head AllToAll bounce buffers (issued as soon as head h is done,
        # so the first exchange overlaps the second head's compute)
        sec = P * SROWS  # elems per (core, head) section
        bounce_in = [dram.tile([NCORES * sec], f16, name=f"bounce_in{h}")
                     for h in range(HPC)]
        bounce_out = [dram.tile([NCORES * sec], f16, name=f"bounce_out{h}")
                      for h in range(HPC)]

        misc_ps = ctx.enter_context(
            tc.tile_pool(name="misc_ps", bufs=1, space="PSUM"))
        VGsl = VG

        with tc.tile_pool(name="expst_p", bufs=2) as expst_pool, \
             tc.tile_pool(name="stA_p", bufs=1, space="PSUM") as stA, \
             tc.tile_pool(name="stB_p", bufs=1, space="PSUM") as stB, \
             tc.tile_pool(name="u_p", bufs=1, space="PSUM") as u_pool, \
             tc.tile_pool(name="cmb", bufs=4) as cmb:

            def emit_scores_exp(h, sc):
                njt = 4 * sc + 4
                expst = [expst_pool.tile([P, njt * SCW], f16,
                                         name=f"exp{hf}", tag=f"exp{hf}",
                                         bufs=2) for hf in range(2)]
                for jp in range(njt // 2):
                    jt0 = 2 * jp
                    # scores for halves A and B run CONCURRENTLY in
                    # disjoint PE row groups (rows 0-63 vs 64-127)
                    psp = [stA.tile([P, 2 * SCW], f32, name="psA", tag="psA"),
                           stB.tile([P, 2 * SCW], f32, name="psB", tag="psB")]
                    for k_ in range(2):
                        jt = jt0 + k_
                        for hf in range(2):
                            rsl = slice(64 * hf, 64 * hf + 64)
                            nc.tensor.matmul(
                                psp[hf][:, k_ * SCW:(k_ + 1) * SCW],
                                kAB[rsl, jt * P:(jt + 1) * P],
                                qkv[h][rsl, sc * SCW:(sc + 1) * SCW],
                                start=True, stop=True)
                    # one exp per pair out of the 2-bank PSUM tile
                    lo = P * (jt0 - 4 * sc) if jt0 >= 4 * sc else 0
                    for hf in range(2):
                        nc.scalar.activation(
                            expst[hf][:, jt0 * SCW + lo:(jt0 + 2) * SCW],
                            psp[hf][:, lo:2 * SCW],
                            mybir.ActivationFunctionType.Exp, scale=SCALE)
                    # causal 0/1 mask on the diagonal tiles
                    for k_ in range(2):
                        jt = jt0 + k_
                        if jt >= 4 * sc:
                            m = jt - 4 * sc
                            lom = P * m
                            if lom < SCW:
                                for hf in range(2):
                                    esl = expst[hf][:, jt * SCW + lom:
                                                    (jt + 1) * SCW]
                                    nc.vector.tensor_mul(
                                        esl, esl,
                                        mask_sb[:, m * SCW + lom:
                                                (m + 1) * SCW])
                return expst

            def emit_av_combine(h, sc, expst):
                njt = 4 * sc + 4
                # AV chains packed 3-per-bank: [q0|q1|q2] x2 + [q3A|q3B];
                # chains sharing a bank are emitted chain-complete, in order,
                # so each start=True only clears bits of finished chains
                ub = [u_pool.tile([P, SCW], f32, name=f"ub{i}",
                                  tag=f"ub{i}") for i in range(3)]

                def uslot(hf, q_):
                    if q_ < 3:
                        return ub[hf], UG2 * q_
                    return ub[2], UG2 * hf

                for hf in range(2):
                    for q_ in range(4):
                        js = 4 * sc + q_
                        put, off = uslot(hf, q_)
                        for jt in range(js + 1):
                            lhs = expst[hf][:, jt * SCW + q_ * P:
                                            jt * SCW + q_ * P + P]
                            nc.tensor.matmul(
                                put[:, off:off + 129],
                                lhs, v16[:, jt * VGsl:jt * VGsl + 129],
                                start=(jt == 0), stop=(jt == js))
                # combine: attn = 0.5*(u1*inv_r1 - lam*u2*inv_r2)
                inv = cmb.tile([P, 8], f32, name="inv", tag="inv")
                for hf in range(2):
                    for q_ in range(4):
                        put, off = uslot(hf, q_)
                        nc.vector.reciprocal(
                            inv[:, 4 * hf + q_:4 * hf + q_ + 1],
                            put[:, off + 128:off + 129])
                for q_ in range(4):
                    js = 4 * sc + q_
                    put0, off0 = uslot(0, q_)
                    put1, off1 = uslot(1, q_)
                    u0 = put0[:, off0:off0 + P]
                    u1 = put1[:, off1:off1 + P]
                    sc2 = cmb.tile([P, 1], f32, name="sc2", tag="sc2")
                    nc.vector.tensor_scalar_mul(
                        sc2[:, :], inv[:, 4 + q_:5 + q_],
                        lam_b[:, h:h + 1])
                    tmp2 = cmb.tile([P, P], f32, name="tmp2", tag="tmp2")
                    nc.vector.tensor_scalar_mul(tmp2[:, :], u1, sc2[:, :])
                    attn_sl = cmb.tile([P, P], f32, name="attn_sl",
                                       tag="attn_sl")
                    nc.vector.scalar_tensor_tensor(
                        attn_sl[:, :], u0,
                        inv[:, q_:q_ + 1], tmp2[:, :],
                        mybir.AluOpType.mult, mybir.AluOpType.subtract)
                    ps_t = misc_ps.tile([P, P], f32, name="ps_at",
                                        tag="misc")
                    nc.tensor.transpose(ps_t[:, :], attn_sl[:, :],
                                        id_sb[:, :])
                    nc.vector.tensor_copy(attnT[h][:, js * P:(js + 1) * P],
                                          ps_t[:, :])
                # ---- Stage 3 (per head): AllToAll into sequence shards
                if sc == NSC - 1:
                    nc.gpsimd.dma_start(
                        out=bounce_in[h][:].rearrange(
                            "(d p f) -> p d f", d=NCORES, f=SROWS),
                        in_=attnT[h][:, :].rearrange(
                            "p (d f) -> p d f", f=SROWS))
                    nc.gpsimd.collective_compute(
                        "AllToAll", mybir.AluOpType.bypass,
                        replica_groups=rg,
                        ins=[bounce_in[h][:]], outs=[bounce_out[h][:]])

            # software pipeline: AV/combine of iteration i-1 runs on the PE
            # while the scalar engine exps iteration i's scores
            prev = None
            for h in range(HPC):
                for sc in range(NSC):
                    expst = emit_scores_exp(h, sc)
                    if prev is not None:
                        emit_av_combine(*prev)
                    prev = (h, sc, expst)
            emit_av_combine(*prev)

        # ---- Stage 4: o_proj over the local 256 rows ----
        with tc.tile_pool(name="aT_p", bufs=1) as aT_pool, \
             tc.tile_pool(name="o_p", bufs=4) as o_pool, \
             tc.tile_pool(name="ps4", bufs=2, space="PSUM") as ps4:
            aTl = []
            for h in range(HPC):
                a_t = aT_pool.tile([P, NCORES * SROWS], f16, name=f"aT{h}")
                nc.gpsimd.dma_start(
                    out=a_t[:, :].rearrange("p (d f) -> p d f", d=NCORES),
                    in_=bounce_out[h][:].rearrange(
                        "(d p f) -> p d f", d=NCORES, f=SROWS))
                aTl.append(a_t)
            # head-0 sections only need the first AllToAll: run ALL of their
            # partial o_proj groups while the second exchange is in flight.
            o_es = {}
            for dc in range(4):
                for st_ in range(2):
                    ps_e = ps4.tile([P, SCW], f32, name="ps_e", tag="oe")
                    for i, ht in enumerate(range(0, H, 2)):
                        nc.tensor.matmul(
                            ps_e[:, :],
                            aTl[0][:, (ht // 2) * SROWS + st_ * P:
                                   (ht // 2) * SROWS + (st_ + 1) * P],
                            wos_l[dc][:, ht * SCW:(ht + 1) * SCW],
                            start=(i == 0), stop=(i == H // 2 - 1))
                    o_e = o_pool.tile([P, SCW], f32, name="o_e",
                                      tag=f"o_e{dc}{st_}", bufs=1)
                    nc.vector.tensor_copy(o_e[:, :], ps_e[:, :])
                    o_es[(dc, st_)] = o_e
            # keep the PE warm across the second AllToAll's wait window;
            # the result is parked in DRAM and never read.
            warm_ps = misc_ps.tile([P, SCW], f32, name="warm", tag="misc")
            for w_ in range(16):
                nc.tensor.matmul(warm_ps[:, :], v16[:, 0:P],
                                 aTl[0][:, 0:SCW], start=True, stop=True)
            warm_sb = o_pool.tile([P, SCW], f32, name="warm_sb",
                                  tag="warm_sb", bufs=1)
            nc.vector.tensor_copy(warm_sb[:, :], warm_ps[:, :])
            warm_dram = dram.tile([P * SCW], f32, name="warm_dram")
            nc.gpsimd.dma_start(
                out=warm_dram[:].rearrange("(p f) -> p f", f=SCW),
                in_=warm_sb[:, :])
            for dc in range(4):
                for st_ in range(2):
                    ps_o = ps4.tile([P, SCW], f32, name="ps_o", tag="o")
                    for i, ht in enumerate(range(1, H, 2)):
                        nc.tensor.matmul(
                            ps_o[:, :],
                            aTl[1][:, (ht // 2) * SROWS + st_ * P:
                                   (ht // 2) * SROWS + (st_ + 1) * P],
                            wos_l[dc][:, ht * SCW:(ht + 1) * SCW],
                            start=(i == 0), stop=(i == H // 2 - 1))
                    o_sb = o_pool.tile([P, SCW], f32, name="o_sb", tag="o_sb")
                    nc.vector.tensor_add(o_sb[:, :], ps_o[:, :],
                                         o_es[(dc, st_)][:, :])
                    nc.sync.dma_start(
                        out=o_out[st_ * P:(st_ + 1) * P,
                                  dc * SCW:(dc + 1) * SCW],
                        in_=o_sb[:, :])

    nc.compile()
    return nc


def _prep(x, freqs_cos, freqs_sin, Wq, Wk, Wv, Wo, lambda_param):
    """Host-side sharding/layout prep. Returns per-core input maps."""
    x2 = np.asarray(x, np.float32).reshape(S, D)
    xT = np.ascontiguousarray(x2.T.astype(np.float16))
    cosT = np.asarray(freqs_cos, np.float32).T   # [64, S]
    sinT = np.asarray(freqs_sin, np.float32).T
    # rows [c0..31 | c0..31 | c32..63 | c32..63] to match the repacked
    # channel layout (half A rows 0..63, half B rows 64..127)
    cosT = np.ascontiguousarray(np.concatenate(
        [cosT[0:32], cosT[0:32], cosT[32:64], cosT[32:64]],
        axis=0).astype(np.float16))
    sinT = np.asarray(sinT, np.float32)
    sinT = np.ascontiguousarray(np.concatenate(
        [sinT[0:32], sinT[0:32], sinT[32:64], sinT[32:64]],
        axis=0).astype(np.float16))
    Wq = np.asarray(Wq, np.float32)
    Wk = np.asarray(Wk, np.float32)
    Wv = np.asarray(Wv, np.float32)
    Wo16 = np.ascontiguousarray(np.asarray(Wo, np.float32).astype(np.float16))
    lamp = np.asarray(lambda_param, np.float32)

    # de-interleave complex pairs, halves contiguous:
    # [x0 p0..31 | x1 p0..31 | x0 p32..63 | x1 p32..63]
    ev = 2 * np.arange(64)
    od = ev + 1
    perm = np.concatenate([ev[0:32], od[0:32], ev[32:64], od[32:64]]
                          ).astype(np.int64)

    # causal mask variants for the 4 in-chunk diagonal positions
    t_rel = np.arange(P)[:, None]
    s_rel = np.arange(SCW)[None, :]
    mask_all = np.empty((P, 4 * SCW), np.float16)
    for m in range(4):
        mask_all[:, m * SCW:(m + 1) * SCW] = np.where(
            P * m + t_rel <= s_rel, 1.0, 0.0)

    ident = np.eye(P, dtype=np.float32)

    in_maps = []
    for c in range(NCORES):
        g = c // 2
        cols = []
        for h in (2 * c, 2 * c + 1):
            cols.append(Wq[:, h * HD:(h + 1) * HD][:, perm])
        cols.append(Wk[:, g * HD:(g + 1) * HD][:, perm])
        cols.append(Wv[:, g * HD:(g + 1) * HD])
        wall = np.ascontiguousarray(
            np.concatenate(cols, axis=1).astype(np.float16))
        in_maps.append({
            "xT": xT,
            "wall": wall,
            "wo": Wo16,
            "cosT": cosT,
            "sinT": sinT,
            "masks": mask_all,
            "ident": ident,
            "lam": np.ascontiguousarray(
                lamp[2 * c:2 * c + 2].reshape(1, HPC)),
        })
    return in_maps


def _run(inputs, trace=False):
    if "nc" not in _CACHE:
        _CACHE["nc"] = _build()
    nc = _CACHE["nc"]
    in_maps = _prep(**inputs)
    res = run_bass_kernel_spmd(nc, in_maps, core_ids=list(range(NCORES)),
                               trace=trace)
    out = np.concatenate([res.results[c]["o_out"] for c in range(NCORES)],
                         axis=0)
    return out.reshape(1, S, D), res


def kernel(**inputs):
    out, _ = _run(inputs)
    return out


# revision 7
# speedup vs baseline: 1.1386x; 1.1386x over previous
"""Differential attention (GQA + RoPE) Bass/Tile kernel for 8 TRN2 NeuronCores.

Sharding: tensor-parallel over the 16 query heads (2 per core, kv head c//2),
Wq/Wk/Wv column-sharded per core; attention output exchanged with an on-device
AllToAll into sequence shards; o_proj row-parallel per sequence shard with the
full Wo on every core; host concatenates the 8 row shards.

Layout notes:
 - x is passed transposed (xT [D, S]) so the contraction dim of every
   projection matmul lands on SBUF partitions.
 - Wq/Wk columns are permuted per head so that score-half A occupies SBUF
   rows 0..63 and half B rows 64..127:
     rows  0..31  x0 of rope pairs  0..31   (half A even components)
     rows 32..63  x1 of rope pairs  0..31
     rows 64..95  x0 of rope pairs 32..63   (half B)
     rows 96..127 x1 of rope pairs 32..63
   With halves contiguous, the two K=64 score matmuls run CONCURRENTLY in
   disjoint PE row-groups (tile_position (0,0) / (64,0)) -> ~2x score rate.
 - Softmax is max-free (scores within +-8, far inside fp16/exp range); the
   row sum rides the AV matmul as a 129th rhs column of 2.0 against the
   loaded exp(score)^T stationary tile; the doubled sum's reciprocal carries
   the 0.5 output scale.
 - exp() is issued once per PAIR of score tiles out of a [128, 1024] 2-bank
   PSUM tile, halving the per-instruction ACT overhead.
 - a1 - lam*a2 is folded linearly: out = (u1*inv_r1 - lam*u2*inv_r2) * 0.5.
"""

import numpy as np
from contextlib import ExitStack

import concourse.bacc as bacc
import concourse.tile as tile
from concourse import mybir
from concourse.bass_utils import run_bass_kernel_spmd

S = 2048
D = 2048
H = 16
KV = 4
HD = 128
HALF = 64
NCORES = 8
HPC = H // NCORES      # 2 query heads per core
P = 128
NT = S // P            # 16 tiles of 128 along s/t
NSC = 4                # s-chunks of 512
SCW = 512
DT = D // P            # 16 tiles along contraction dim
SROWS = S // NCORES    # 256 output rows per core
SCALE = 1.0 / 8.0      # 1/sqrt(HALF)
OUT_SCALE = 0.5        # 1 - lambda_init
UG2 = 160              # AV chain stride inside a shared PSUM bank (fp32 elems)

f32 = mybir.dt.float32
f16 = mybir.dt.float16

_CACHE = {}


def _build():
    nc = bacc.Bacc("TRN2", target_bir_lowering=False, debug=False,
                   num_devices=NCORES)
    xT = nc.declare_dram_parameter("xT", [D, S], f16, isOutput=False)
    wall = nc.declare_dram_parameter("wall", [D, 4 * P], f16, isOutput=False)
    wo = nc.declare_dram_parameter("wo", [D, D], f16, isOutput=False)
    cosT = nc.declare_dram_parameter("cosT", [P, S], f16, isOutput=False)
    sinT = nc.declare_dram_parameter("sinT", [P, S], f16, isOutput=False)
    masks = nc.declare_dram_parameter("masks", [P, 4 * SCW], f16, isOutput=False)
    ident = nc.declare_dram_parameter("ident", [P, P], f32, isOutput=False)
    lam = nc.declare_dram_parameter("lam", [1, HPC], f32, isOutput=False)
    o_out = nc.declare_dram_parameter("o_out", [SROWS, D], f32, isOutput=True)

    rg = [list(range(NCORES))]

    with tile.TileContext(nc) as tc, ExitStack() as ctx:
        const = ctx.enter_context(tc.tile_pool(name="const", bufs=1))
        dram = ctx.enter_context(tc.tile_pool(name="dram", bufs=1, space="DRAM"))

        # cos/sin rows follow the repacked channel layout:
        # [c0..31 | c0..31 | c32..63 | c32..63]
        cos_sb = const.tile([P, S], f16)
        nc.gpsimd.dma_start(out=cos_sb[:, :], in_=cosT[:, :])
        sin_sb = const.tile([P, S], f16)
        nc.gpsimd.dma_start(out=sin_sb[:, :], in_=sinT[:, :])
        mask_sb = const.tile([P, 4 * SCW], f16)
        nc.gpsimd.dma_start(out=mask_sb[:, :], in_=masks[:, :])
        id_sb = const.tile([P, P], f32)
        nc.gpsimd.dma_start(out=id_sb[:, :], in_=ident[:, :])
        lam_sb = const.tile([1, HPC], f32)
        nc.gpsimd.dma_start(out=lam_sb[:, :], in_=lam[:, :])
        lam_sig = const.tile([1, HPC], f32)
        nc.scalar.activation(lam_sig[:, :], lam_sb[:, :],
                             mybir.ActivationFunctionType.Sigmoid)
        lam_b = const.tile([P, HPC], f32)
        nc.gpsimd.partition_broadcast(lam_b[:, :], lam_sig[:, :])

        # Persistent per-core tensors: q per head (rope applied, half-A rows
        # 0..63 / half-B rows 64..127), k likewise in kAB, vT (fp32
        # channel-major) and v16 (fp16 t-major for the AV rhs).
        qkvp = ctx.enter_context(tc.tile_pool(name="qkvp", bufs=1))
        qkv = [qkvp.tile([P, S], f16, name=f"qkv{j}") for j in range(2)]
        kAB = qkvp.tile([P, S], f16, name="kAB")
        vT32 = qkvp.tile([P, S], f32)
        # v in t-major fp16, one 136-wide group per t-tile:
        # cols [136jt, 136jt+128) = v, col 136jt+128 = 2.0 -- the rowsum
        # rider; pre-doubled sums make their reciprocal carry the 0.5 scale
        VG = 136
        v16 = qkvp.tile([P, NT * VG], f16)
        nc.vector.memset(v16[:, :].rearrange("p (jt g) -> p jt g", g=VG)
                         [:, :, 128:129], 2.0)

        # ---- Stage 1: fused qkv projection (+RoPE on eviction) ----
        with tc.tile_pool(name="wall_p", bufs=1) as wall_pool, \
             tc.tile_pool(name="xt_p", bufs=2) as xt_pool, \
             tc.tile_pool(name="rtmp", bufs=4) as rtmp, \
             tc.tile_pool(name="qscr", bufs=6) as qscr, \
             tc.tile_pool(name="ps1", bufs=2, space="PSUM") as ps1:
            # weight block split into 4 dt-groups spread over 4 DMA queues so
            # the first group lands quickly and matmuls can start early
            w_sb = wall_pool.tile([P, DT * 4 * P], f16, name="w_sb")
            # weight dt-groups on the scalar queue (x rides sync) so group 0
            # of both lands after ~0.5MB each and matmuls start early
            for g in range(4):
                gdt = slice(g * 4, (g + 1) * 4)
                nc.scalar.dma_start(
                    out=w_sb[:, g * 4 * 4 * P:(g + 1) * 4 * 4 * P].rearrange(
                        "p (dt j) -> p dt j", dt=4),
                    in_=wall.ap().rearrange("(dt p) j -> p dt j", p=P)
                    [:, gdt, :])

            for sc in range(NSC):
                # x chunk in 4 dt-groups, all on the sync queue (w rides the
                # scalar queue) so group g lands while group g-1 computes
                xts = xt_pool.tile([P, DT * SCW], f16, name="xt", tag="xt")
                for g in range(4):
                    gdt = slice(g * 4, (g + 1) * 4)
                    nc.sync.dma_start(
                        out=xts[:, g * 4 * SCW:(g + 1) * 4 * SCW].rearrange(
                            "p (dt f) -> p dt f", dt=4),
                        in_=xT[:, sc * SCW:(sc + 1) * SCW].rearrange(
                            "(dt p) f -> p dt f", p=P)[:, gdt, :])
                # 4 j-chains live simultaneously (4 banks x bufs=2); dt-groups
                # interleave so compute starts after the first 1MB arrives
                pst = [ps1.tile([P, SCW], f32, name=f"p{j}", tag=f"p{j}")
                       for j in range(4)]
                for g in range(4):
                    for j in (2, 3, 0, 1):
                        for dt_ in range(4 * g, 4 * g + 4):
                            nc.tensor.matmul(
                                pst[j][:, :],
                                w_sb[:, dt_ * 4 * P + j * P:
                                     dt_ * 4 * P + (j + 1) * P],
                                xts[:, dt_ * SCW:(dt_ + 1) * SCW],
                                start=(dt_ == 0), stop=(dt_ == DT - 1))
                ssl = slice(sc * SCW, (sc + 1) * SCW)
                for j in (2, 0, 1, 3):
                    psum_p = pst[j]
                    tags = ("t0", "t1") if j < 2 else ("kt0", "kt1")
                    t0 = rtmp.tile([P, SCW], f16, name="t0", tag=tags[0])
                    t1 = rtmp.tile([P, SCW], f16, name="t1", tag=tags[1])
                    if j < 3:
                        # evict on the scalar engine so the PSUM bank frees
                        # fast; rope then runs SBUF->SBUF on DVE
                        xsc = qscr.tile([P, SCW], f16, name="xsc", tag="xsc")
                        nc.scalar.copy(xsc[:, :], psum_p[:, :])
                        psum_p = xsc
                        dst = qkv[j] if j < 2 else kAB
                        # rope per half: rows [b..b+32) = x0*cos - x1*sin,
                        # rows [b+32..b+64) = x0*sin + x1*cos
                        for b in (0, 64):
                            r0 = slice(b, b + 32)
                            r1 = slice(b + 32, b + 64)
                            nc.vector.tensor_mul(t0[r0, :], psum_p[r1, :],
                                                 sin_sb[r1, ssl])
                            nc.vector.tensor_mul(dst[r0, ssl], psum_p[r0, :],
                                                 cos_sb[r0, ssl])
                            nc.vector.tensor_sub(dst[r0, ssl], dst[r0, ssl],
                                                 t0[r0, :])
                            nc.vector.tensor_mul(t1[r1, :], psum_p[r0, :],
                                                 sin_sb[r0, ssl])
                            nc.vector.tensor_mul(dst[r1, ssl], psum_p[r1, :],
                                                 cos_sb[r1, ssl])
                            nc.vector.tensor_add(dst[r1, ssl], dst[r1, ssl],
                                                 t1[r1, :])
                    else:
                        nc.scalar.copy(vT32[:, ssl], psum_p[:, :])

        # ---- Stage 1.5: transpose v to t-major fp16 ----
        with tc.tile_pool(name="pst", bufs=1, space="PSUM") as pstp:
            for jt in range(NT):
                ps_t = pstp.tile([P, P], f32, name="ps_vt", tag="vt")
                nc.tensor.transpose(ps_t[:, :],
                                    vT32[:, jt * P:(jt + 1) * P], id_sb[:, :])
                nc.scalar.copy(v16[:, jt * VG:jt * VG + P], ps_t[:, :])

        # prefetch full Wo (fp16) early on the gpsimd DMA queue so the
        # o_proj partials can start the moment the first AllToAll lands
        wo_pool = ctx.enter_context(tc.tile_pool(name="wo_p", bufs=1))
        wos_l = []
        for dc in range(4):
            wos = wo_pool.tile([P, H * SCW], f16, name="wos", tag=f"wos{dc}")
            nc.gpsimd.dma_start(
                out=wos[:, :].rearrange("p (ht f) -> p ht f", ht=H),
                in_=wo[:, dc * SCW:(dc + 1) * SCW].rearrange(
                    "(ht p) f -> p ht f", p=P))
            wos_l.append(wos)

        # ---- Stage 2: differential attention per (head, s-chunk) ----
        attnp = ctx.enter_context(tc.tile_pool(name="attnp", bufs=1))
        attnT = [attnp.tile([P, S], f16, name=f"attnT{h}") for h in range(HPC)]

        # per-head AllToAll bounce buffers (issued as soon as head h is done,
        # so the first exchange overlaps the second head's compute)
        sec = P * SROWS  # elems per (core, head) section
        bounce_in = [dram.tile([NCORES * sec], f16, name=f"bounce_in{h}")
                     for h in range(HPC)]
        bounce_out = [dram.tile([NCORES * sec], f16, name=f"bounce_out{h}")
                      for h in range(HPC)]

        misc_ps = ctx.enter_context(
            tc.tile_pool(name="misc_ps", bufs=1, space="PSUM"))
        VGsl = VG

        with tc.tile_pool(name="expst_p", bufs=2) as expst_pool, \
             tc.tile_pool(name="stA_p", bufs=1, space="PSUM") as stA, \
             tc.tile_pool(name="stB_p", bufs=1, space="PSUM") as stB, \
             tc.tile_pool(name="u_p", bufs=1, space="PSUM") as u_pool, \
             tc.tile_pool(name="cmb", bufs=4) as cmb:

            def make_pair_emitters(h, sc, expst):
                njt = 4 * sc + 4

                def emit_pair(jp):
                    jt0 = 2 * jp
                    # scores for halves A and B target disjoint PE row
                    # groups (rows 0-63 vs 64-127)
                    psp = [stA.tile([P, 2 * SCW], f32, name="psA", tag="psA"),
                           stB.tile([P, 2 * SCW], f32, name="psB", tag="psB")]
                    for k_ in range(2):
                        jt = jt0 + k_
                        for hf in range(2):
                            rsl = slice(64 * hf, 64 * hf + 64)
                            nc.tensor.matmul(
                                psp[hf][:, k_ * SCW:(k_ + 1) * SCW],
                                kAB[rsl, jt * P:(jt + 1) * P],
                                qkv[h][rsl, sc * SCW:(sc + 1) * SCW],
                                start=True, stop=True)
                    # one exp per pair out of the 2-bank PSUM tile
                    lo = P * (jt0 - 4 * sc) if jt0 >= 4 * sc else 0
                    for hf in range(2):
                        nc.scalar.activation(
                            expst[hf][:, jt0 * SCW + lo:(jt0 + 2) * SCW],
                            psp[hf][:, lo:2 * SCW],
                            mybir.ActivationFunctionType.Exp, scale=SCALE)
                    # causal 0/1 mask on the diagonal tiles
                    for k_ in range(2):
                        jt = jt0 + k_
                        if jt >= 4 * sc:
                            m = jt - 4 * sc
                            lom = P * m
                            if lom < SCW:
                                for hf in range(2):
                                    esl = expst[hf][:, jt * SCW + lom:
                                                    (jt + 1) * SCW]
                                    nc.vector.tensor_mul(
                                        esl, esl,
                                        mask_sb[:, m * SCW + lom:
                                                (m + 1) * SCW])
                return [lambda jp=jp: emit_pair(jp) for jp in range(njt // 2)]

            def av_combine_steps(h, sc, expst):
                """Generator of emission steps: 8 AV chains, then combine
                (+AllToAll). AV chains pack 3-per-bank; chains sharing a
                bank are emitted chain-complete, in order, so each
                start=True only clears bits of finished chains."""
                njt = 4 * sc + 4
                ub = [u_pool.tile([P, SCW], f32, name=f"ub{i}",
                                  tag=f"ub{i}") for i in range(3)]

                def uslot(hf, q_):
                    if q_ < 3:
                        return ub[hf], UG2 * q_
                    return ub[2], UG2 * hf

                for hf in range(2):
                    for q_ in range(4):
                        js = 4 * sc + q_
                        put, off = uslot(hf, q_)
                        for jt in range(js + 1):
                            lhs = expst[hf][:, jt * SCW + q_ * P:
                                            jt * SCW + q_ * P + P]
                            nc.tensor.matmul(
                                put[:, off:off + 129],
                                lhs, v16[:, jt * VGsl:jt * VGsl + 129],
                                start=(jt == 0), stop=(jt == js))
                        yield
                # combine: attn = 0.5*(u1*inv_r1 - lam*u2*inv_r2)
                inv = cmb.tile([P, 8], f32, name="inv", tag="inv")
                for hf in range(2):
                    for q_ in range(4):
                        put, off = uslot(hf, q_)
                        nc.vector.reciprocal(
                            inv[:, 4 * hf + q_:4 * hf + q_ + 1],
                            put[:, off + 128:off + 129])
                for q_ in range(4):
                    js = 4 * sc + q_
                    put0, off0 = uslot(0, q_)
                    put1, off1 = uslot(1, q_)
                    u0 = put0[:, off0:off0 + P]
                    u1 = put1[:, off1:off1 + P]
                    sc2 = cmb.tile([P, 1], f32, name="sc2", tag="sc2")
                    nc.vector.tensor_scalar_mul(
                        sc2[:, :], inv[:, 4 + q_:5 + q_],
                        lam_b[:, h:h + 1])
                    tmp2 = cmb.tile([P, P], f32, name="tmp2", tag="tmp2")
                    nc.vector.tensor_scalar_mul(tmp2[:, :], u1, sc2[:, :])
                    attn_sl = cmb.tile([P, P], f32, name="attn_sl",
                                       tag="attn_sl")
                    nc.vector.scalar_tensor_tensor(
                        attn_sl[:, :], u0,
                        inv[:, q_:q_ + 1], tmp2[:, :],
                        mybir.AluOpType.mult, mybir.AluOpType.subtract)
                    ps_t = misc_ps.tile([P, P], f32, name="ps_at",
                                        tag="misc")
                    nc.tensor.transpose(ps_t[:, :], attn_sl[:, :],
                                        id_sb[:, :])
                    nc.vector.tensor_copy(attnT[h][:, js * P:(js + 1) * P],
                                          ps_t[:, :])
                    yield
                # ---- Stage 3 (per head): AllToAll into sequence shards
                if sc == NSC - 1:
                    nc.gpsimd.dma_start(
                        out=bounce_in[h][:].rearrange(
                            "(d p f) -> p d f", d=NCORES, f=SROWS),
                        in_=attnT[h][:, :].rearrange(
                            "p (d f) -> p d f", f=SROWS))
                    nc.gpsimd.collective_compute(
                        "AllToAll", mybir.AluOpType.bypass,
                        replica_groups=rg,
                        ins=[bounce_in[h][:]], outs=[bounce_out[h][:]])
                yield

            # software pipeline: the AV/combine steps of iteration i-1 are
            # interleaved BETWEEN iteration i's score pairs so the PE has
            # work while the scalar engine exps each pair
            prev_steps = None
            for h in range(HPC):
                for sc in range(NSC):
                    njt = 4 * sc + 4
                    expst = [expst_pool.tile([P, njt * SCW], f16,
                                             name=f"exp{hf}", tag=f"exp{hf}",
                                             bufs=2) for hf in range(2)]
                    pairs = make_pair_emitters(h, sc, expst)
                    nsteps = 13  # 8 chains + 4 combine + 1 a2a
                    done = 0
                    for ip, pe_ in enumerate(pairs):
                        pe_()
                        if prev_steps is not None:
                            want = (ip + 1) * nsteps // len(pairs)
                            while done < want:
                                if next(prev_steps, "END") == "END":
                                    done = nsteps
                                    break
                                done += 1
                    if prev_steps is not None:
                        for _ in prev_steps:
                            pass
                    prev_steps = av_combine_steps(h, sc, expst)
            for _ in prev_steps:
                pass

        # ---- Stage 4: o_proj over the local 256 rows ----
        with tc.tile_pool(name="aT_p", bufs=1) as aT_pool, \
             tc.tile_pool(name="o_p", bufs=4) as o_pool, \
             tc.tile_pool(name="ps4", bufs=2, space="PSUM") as ps4:
            aTl = []
            for h in range(HPC):
                a_t = aT_pool.tile([P, NCORES * SROWS], f16, name=f"aT{h}")
                nc.gpsimd.dma_start(
                    out=a_t[:, :].rearrange("p (d f) -> p d f", d=NCORES),
                    in_=bounce_out[h][:].rearrange(
                        "(d p f) -> p d f", d=NCORES, f=SROWS))
                aTl.append(a_t)
            # head-0 sections only need the first AllToAll: run ALL of their
            # partial o_proj groups while the second exchange is in flight.
            o_es = {}
            for dc in range(4):
                for st_ in range(2):
                    ps_e = ps4.tile([P, SCW], f32, name="ps_e", tag="oe")
                    for i, ht in enumerate(range(0, H, 2)):
                        nc.tensor.matmul(
                            ps_e[:, :],
                            aTl[0][:, (ht // 2) * SROWS + st_ * P:
                                   (ht // 2) * SROWS + (st_ + 1) * P],
                            wos_l[dc][:, ht * SCW:(ht + 1) * SCW],
                            start=(i == 0), stop=(i == H // 2 - 1))
                    o_e = o_pool.tile([P, SCW], f32, name="o_e",
                                      tag=f"o_e{dc}{st_}", bufs=1)
                    nc.vector.tensor_copy(o_e[:, :], ps_e[:, :])
                    o_es[(dc, st_)] = o_e
            # keep the PE warm across the second AllToAll's wait window;
            # the result is parked in DRAM and never read.
            warm_ps = misc_ps.tile([P, SCW], f32, name="warm", tag="misc")
            for w_ in range(16):
                nc.tensor.matmul(warm_ps[:, :], v16[:, 0:P],
                                 aTl[0][:, 0:SCW], start=True, stop=True)
            warm_sb = o_pool.tile([P, SCW], f32, name="warm_sb",
                                  tag="warm_sb", bufs=1)
            nc.vector.tensor_copy(warm_sb[:, :], warm_ps[:, :])
            warm_dram = dram.tile([P * SCW], f32, name="warm_dram")
            nc.gpsimd.dma_start(
                out=warm_dram[:].rearrange("(p f) -> p f", f=SCW),
                in_=warm_sb[:, :])
            for dc in range(4):
                for st_ in range(2):
                    ps_o = ps4.tile([P, SCW], f32, name="ps_o", tag="o")
                    for i, ht in enumerate(range(1, H, 2)):
                        nc.tensor.matmul(
                            ps_o[:, :],
                            aTl[1][:, (ht // 2) * SROWS + st_ * P:
                                   (ht // 2) * SROWS + (st_ + 1) * P],
                            wos_l[dc][:, ht * SCW:(ht + 1) * SCW],
                            start=(i == 0), stop=(i == H // 2 - 1))
                    o_sb = o_pool.tile([P, SCW], f32, name="o_sb", tag="o_sb")
                    nc.vector.tensor_add(o_sb[:, :], ps_o[:, :],
                                         o_es[(dc, st_)][:, :])
                    nc.sync.dma_start(
                        out=o_out[st_ * P:(st_ + 1) * P,
                                  dc * SCW:(dc + 1) * SCW],
                        in_=o_sb[:, :])

    nc.compile()
    return nc


def _prep(x, freqs_cos, freqs_sin, Wq, Wk, Wv, Wo, lambda_param):
    """Host-side sharding/layout prep. Returns per-core input maps."""
    x2 = np.asarray(x, np.float32).reshape(S, D)
    xT = np.ascontiguousarray(x2.T.astype(np.float16))
    cosT = np.asarray(freqs_cos, np.float32).T   # [64, S]
    sinT = np.asarray(freqs_sin, np.float32).T
    # rows [c0..31 | c0..31 | c32..63 | c32..63] to match the repacked
    # channel layout (half A rows 0..63, half B rows 64..127)
    cosT = np.ascontiguousarray(np.concatenate(
        [cosT[0:32], cosT[0:32], cosT[32:64], cosT[32:64]],
        axis=0).astype(np.float16))
    sinT = np.asarray(sinT, np.float32)
    sinT = np.ascontiguousarray(np.concatenate(
        [sinT[0:32], sinT[0:32], sinT[32:64], sinT[32:64]],
        axis=0).astype(np.float16))
    Wq = np.asarray(Wq, np.float32)
    Wk = np.asarray(Wk, np.float32)
    Wv = np.asarray(Wv, np.float32)
    Wo16 = np.ascontiguousarray(np.asarray(Wo, np.float32).astype(np.float16))
    lamp = np.asarray(lambda_param, np.float32)

    # de-interleave complex pairs, halves contiguous:
    # [x0 p0..31 | x1 p0..31 | x0 p32..63 | x1 p32..63]
    ev = 2 * np.arange(64)
    od = ev + 1
    perm = np.concatenate([ev[0:32], od[0:32], ev[32:64], od[32:64]]
                          ).astype(np.int64)

    # causal mask variants for the 4 in-chunk diagonal positions
    t_rel = np.arange(P)[:, None]
    s_rel = np.arange(SCW)[None, :]
    mask_all = np.empty((P, 4 * SCW), np.float16)
    for m in range(4):
        mask_all[:, m * SCW:(m + 1) * SCW] = np.where(
            P * m + t_rel <= s_rel, 1.0, 0.0)

    ident = np.eye(P, dtype=np.float32)

    in_maps = []
    for c in range(NCORES):
        g = c // 2
        cols = []
        for h in (2 * c, 2 * c + 1):
            cols.append(Wq[:, h * HD:(h + 1) * HD][:, perm])
        cols.append(Wk[:, g * HD:(g + 1) * HD][:, perm])
        cols.append(Wv[:, g * HD:(g + 1) * HD])
        wall = np.ascontiguousarray(
            np.concatenate(cols, axis=1).astype(np.float16))
        in_maps.append({
            "xT": xT,
            "wall": wall,
            "wo": Wo16,
            "cosT": cosT,
            "sinT": sinT,
            "masks": mask_all,
            "ident": ident,
            "lam": np.ascontiguousarray(
                lamp[2 * c:2 * c + 2].reshape(1, HPC)),
        })
    return in_maps


def _run(inputs, trace=False):
    if "nc" not in _CACHE:
        _CACHE["nc"] = _build()
    nc = _CACHE["nc"]
    in_maps = _prep(**inputs)
    res = run_bass_kernel_spmd(nc, in_maps, core_ids=list(range(NCORES)),
                               trace=trace)
    out = np.concatenate([res.results[c]["o_out"] for c in range(NCORES)],
                         axis=0)
    return out.reshape(1, S, D), res


def kernel(**inputs):
    out, _ = _run(inputs)
    return out
